# revision 4
# baseline (speedup 1.0000x reference)
"""BinarizedLinear on 8 Trainium2 NeuronCores.

out = x @ sign(weight).T + bias
  x: (32768, 1024) f32, weight: (1024, 1024) f32, bias: (1024,) f32

Strategy (data-parallel over batch, weight/bias replicated):
  - each core handles a 4096-row shard of x
  - contraction K=1024 split 256 + 768: the first 256 features ride as
    fp8 pairs through ONE DoubleRow matmul (2 fp8 MACs/cell/cycle, the
    +-1 weight is exact in fp8; fp8 rounding of x costs ~1.3e-2 rel
    err, inside the 2e-2 gate), the remaining 768 features go as six
    bf16 matmuls -- 7 matmuls per PSUM group instead of 8
  - host marshals x to [p, su, (ic|j), b] tiled layouts (p = feature %
    128 -> SBUF partition, su = batch/128 tile, b = batch % 128) so
    each batch window is one DMA with multi-KB contiguous per-partition
    segments while every matmul stationary tile stays a contiguous
    per-partition slice (FWL stays enabled for the bf16 tiles)
  - weight order: fp8 pair-weights (256KB) lead the sync queue, then x
    windows; bf16-side weights split per output half (2x 384KB) lead
    the scalar queue so the first matmuls are never gated by a 1MB DMA;
    bias broadcast rides scalar after the weights (DVE slack absorbs)
  - device: PE matmul (x stationary, K accumulated in PSUM, N=512) ->
    DVE bias-add writing bf16 -> contiguous 256KB store (scalar queue)
  - output returned as bf16 [4096, 1024]; host upcasts to f32
  - x windows ramp 128..2048 batch rows, all enqueued up front; a small
    warmup burst un-throttles the PE clock (HAM) during the fill
"""

import os
import sys

import numpy as np

sys.path.insert(0, "/opt/trn_rl_repo")

import ml_dtypes

import concourse.tile as tile
from concourse import bacc, mybir
from concourse.bass_utils import run_bass_kernel_spmd

N_CORES = 8
B_FULL = 32768
I_DIM = 1024
O_DIM = 1024
BS = B_FULL // N_CORES  # 4096 batch rows per core

P = 128                # partitions / contraction tile
KF8 = 2                # leading k-chunks in fp8 via DoubleRow
KBF = I_DIM // P - KF8  # remaining k-chunks in bf16 (6)
N_OC = 512             # psum free width (one PSUM bank of f32)
OC = O_DIM // N_OC     # 2 output chunks
B_SUB = 128            # stationary-operand free width (psum partitions)
N_SU = BS // B_SUB     # 32 batch tiles per core
WINDOWS = [1, 1, 2, 4, 8, 16]  # batch windows in su units
assert sum(WINDOWS) == N_SU
N_WARM = 4
SU_WB = B_SUB * KBF    # bf16 elements per su per partition (768)
SU_W8 = B_SUB * KF8    # fp8 elements per su per partition (256)

F32 = mybir.dt.float32
BF16 = mybir.dt.bfloat16
FP8 = mybir.dt.float8e4
DR = mybir.MatmulPerfMode.DoubleRow

_cache = {}


def _build_program():
    nc = bacc.Bacc("TRN2", target_bir_lowering=False, debug=False,
                   num_devices=N_CORES)

    xt = nc.dram_tensor("xt", [P, N_SU * SU_WB], BF16,
                        kind="ExternalInput").ap()
    xt8 = nc.dram_tensor("xt8", [P, N_SU, KF8, B_SUB], FP8,
                         kind="ExternalInput").ap()
    w8 = nc.dram_tensor("w8", [P, KF8, O_DIM], FP8,
                        kind="ExternalInput").ap()
    wt0 = nc.dram_tensor("wt0", [P, KBF * N_OC], FP8,
                         kind="ExternalInput").ap()
    wt1 = nc.dram_tensor("wt1", [P, KBF * N_OC], FP8,
                         kind="ExternalInput").ap()
    bias_d = nc.dram_tensor("bias_d", [1, O_DIM], F32,
                            kind="ExternalInput").ap()
    out = nc.dram_tensor("out", [BS, O_DIM], BF16, kind="ExternalOutput").ap()

    with tile.TileContext(nc) as tc:
        with (
            tc.tile_pool(name="consts", bufs=1) as consts,
            tc.tile_pool(name="xb", bufs=1) as xb_pool,
            tc.tile_pool(name="ot", bufs=8) as ot_pool,
            tc.tile_pool(name="ps", bufs=6, space="PSUM") as ps_pool,
        ):
            # PE warmup: data-independent matmuls on scratch SBUF keep the
            # PE busy through the first window's DMA fill so HAM
            # un-throttles to 2.4 GHz before the first real matmul.
            warm_sc = consts.tile([P, N_OC], BF16)
            nc.vector.memset(warm_sc[:], 0.0)
            ps_w = ps_pool.tile([P, N_OC], F32, tag="warm", bufs=1)
            for _ in range(N_WARM):
                nc.tensor.matmul(ps_w[:], warm_sc[:, :B_SUB], warm_sc[:],
                                 start=True, stop=True, skip_group_check=True)

            # fp8 pair-weights lead the sync queue (first matmul needs them)
            w8_sb = consts.tile([P, KF8, O_DIM], FP8)
            nc.sync.dma_start(w8_sb[:, :, :], w8[:, :, :])

            # bf16-side weights, one DMA per output half, then bias, on the
            # scalar queue
            wt_sb = []
            for oc, wsrc in enumerate((wt0, wt1)):
                w_t = consts.tile([P, KBF * N_OC], FP8, tag=f"wt{oc}")
                nc.scalar.dma_start(w_t[:], wsrc[:, :])
                wt_sb.append(w_t)
            bias_sb = consts.tile([P, O_DIM], F32)
            nc.scalar.dma_start(bias_sb[:],
                                bias_d[0, :].partition_broadcast(P))

            # x windows: fp8 pairs then bf16, enqueued up front on sync.
            off = [0]
            for w in WINDOWS:
                off.append(off[-1] + w)
            xw8, xwb = [], []
            for wi, w in enumerate(WINDOWS):
                s0 = off[wi]
                x8s = xb_pool.tile([P, w, KF8, B_SUB], FP8, tag=f"x8_{wi}",
                                   bufs=1)
                nc.sync.dma_start(x8s[:, :, :, :], xt8[:, s0:s0 + w, :, :])
                xbs = xb_pool.tile([P, w * SU_WB], BF16, tag=f"xb_{wi}",
                                   bufs=1)
                nc.sync.dma_start(xbs[:], xt[:, s0 * SU_WB:(s0 + w) * SU_WB])
                xw8.append(x8s)
                xwb.append(xbs)

            for wi, w in enumerate(WINDOWS):
                s0 = off[wi]
                for lsu in range(w):
                    su = s0 + lsu
                    r0 = su * B_SUB
                    ot = ot_pool.tile([P, O_DIM], BF16, tag="ot")
                    for oc in range(OC):
                        ps = ps_pool.tile([P, N_OC], F32, tag="ps")
                        # k-chunks 0-1 in one double-pumped fp8 matmul
                        nc.tensor.matmul(
                            ps[:],
                            xw8[wi][:, lsu, :, :],
                            w8_sb[:, :, oc * N_OC:(oc + 1) * N_OC],
                            start=True, stop=False, perf_mode=DR)
                        # k-chunks 2-7 in bf16
                        for kk in range(KBF):
                            nc.tensor.matmul(
                                ps[:],
                                xwb[wi][:, lsu * SU_WB + kk * B_SUB:
                                        lsu * SU_WB + kk * B_SUB + B_SUB],
                                wt_sb[oc][:, kk * N_OC:(kk + 1) * N_OC],
                                start=False,
                                stop=(kk == KBF - 1),
                            )
                        nc.vector.tensor_add(
                            ot[:, oc * N_OC:(oc + 1) * N_OC], ps[:],
                            bias_sb[:, oc * N_OC:(oc + 1) * N_OC])
                    # 256KB fully-contiguous bf16 store of 128 output rows.
                    nc.scalar.dma_start(out[r0:r0 + B_SUB, :], ot[:])

    nc.compile()
    return nc


def _get_program():
    if "prog" not in _cache:
        _cache["prog"] = _build_program()
    return _cache["prog"]


def _marshal_w(weight: np.ndarray):
    s = np.sign(weight)
    s[s == 0] = 1.0
    st = s.T  # [i, o]
    # fp8 pair-weights for k-chunks 0-1: [p, j, o]
    w8 = np.ascontiguousarray(
        st[:KF8 * P].reshape(KF8, P, O_DIM).transpose(1, 0, 2)).astype(
        ml_dtypes.float8_e4m3)
    # bf16-side weights (chunks 2-7), split per output half: [p, kk, o']
    w3 = st[KF8 * P:].reshape(KBF, P, O_DIM)
    wt0 = np.ascontiguousarray(
        w3[:, :, :N_OC].transpose(1, 0, 2)).reshape(
        P, KBF * N_OC).astype(ml_dtypes.float8_e4m3)
    wt1 = np.ascontiguousarray(
        w3[:, :, N_OC:].transpose(1, 0, 2)).reshape(
        P, KBF * N_OC).astype(ml_dtypes.float8_e4m3)
    return w8, wt0, wt1


def _marshal_x(x_shard: np.ndarray):
    # [B, F] -> [su, b, (j|ic), p] -> [p, su, (j|ic), b]
    x8 = x_shard[:, :KF8 * P].reshape(N_SU, B_SUB, KF8, P).transpose(
        3, 0, 2, 1)
    x8 = np.ascontiguousarray(x8).astype(ml_dtypes.float8_e4m3)
    xb = x_shard[:, KF8 * P:].reshape(N_SU, B_SUB, KBF, P).transpose(
        3, 0, 2, 1)
    xb = np.ascontiguousarray(xb).astype(ml_dtypes.bfloat16).reshape(
        P, N_SU * SU_WB)
    return x8, xb


def kernel_impl(x, weight, bias, mode=None, trace=False, tmpdir=None):
    w8, wt0, wt1 = _marshal_w(np.asarray(weight))
    bias_d = np.ascontiguousarray(np.asarray(bias, np.float32)[None, :])
    x = np.asarray(x, np.float32)

    in_maps = []
    for c in range(N_CORES):
        x8, xb = _marshal_x(x[c * BS:(c + 1) * BS])
        in_maps.append({"xt": xb, "xt8": x8, "w8": w8, "wt0": wt0,
                        "wt1": wt1, "bias_d": bias_d})

    nc = _get_program()
    try:
        res = run_bass_kernel_spmd(nc, in_maps, list(range(N_CORES)),
                                   trace=trace, tmpdir=tmpdir)
    except Exception:
        # transient runtime hiccups (e.g. first dispatch after long idle)
        res = run_bass_kernel_spmd(nc, in_maps, list(range(N_CORES)),
                                   trace=trace, tmpdir=tmpdir)
    out = np.concatenate(
        [np.asarray(res.results[c]["out"]).astype(np.float32)
         for c in range(N_CORES)], axis=0)
    return out, res


def kernel(x, weight, bias):
    out, _ = kernel_impl(x, weight, bias)
    return out


# revision 5
# speedup vs baseline: 1.0589x; 1.0589x over previous
"""BinarizedLinear on 8 Trainium2 NeuronCores.

out = x @ sign(weight).T + bias
  x: (32768, 1024) f32, weight: (1024, 1024) f32, bias: (1024,) f32

Strategy (data-parallel over batch, weight/bias replicated):
  - each core handles a 4096-row shard of x
  - host marshals the shard to bf16 in a [p, su, ic, b] tiled layout
    (p = feature % 128 -> SBUF partition, su = batch/128 tile, ic =
    feature/128 contraction chunk, b = batch % 128) so that each batch
    window is ONE DMA with multi-KB contiguous per-partition segments
    (large descriptors -> near-peak HBM bandwidth) while every matmul
    stationary tile xs[.., ic*128:+128] stays a contiguous
    256B-per-partition slice (fast weight load stays enabled)
  - the binarized +-1 weight is exact in fp8, host-packed [p, ic, o']
    per output half and shipped as one 384KB DMA per half -- the first
    half leads the sync queue ahead of x so the first matmul group is
    never gated by a monolithic weight load; the second half plus the
    bias broadcast lead the scalar queue ahead of the output stores
  - device: PE matmul (x tile stationary, K=1024 accumulated in PSUM
    over 8 chunks, N=512 free) -> DVE bias-add writing bf16 ->
    contiguous 256KB store (scalar queue)
  - output returned as bf16 [4096, 1024]; host upcasts to f32
  - x windows ramp 128..2048 batch rows, all enqueued up front on the
    sync queue so DMA runs far ahead of the PE; a warmup burst
    un-throttles the PE clock (HAM) during the first window's fill
"""

import os
import sys

import numpy as np

sys.path.insert(0, "/opt/trn_rl_repo")

import ml_dtypes

import concourse.tile as tile
from concourse import bacc, mybir
from concourse.bass_utils import run_bass_kernel_spmd

N_CORES = 8
B_FULL = 32768
I_DIM = 1024
O_DIM = 1024
BS = B_FULL // N_CORES  # 4096 batch rows per core

P = 128                # partitions / contraction tile
IC = I_DIM // P        # 8 contraction chunks
N_OC = 512             # psum free width (one PSUM bank of f32)
OC = O_DIM // N_OC     # 2 output chunks
B_SUB = 128            # stationary-operand free width (psum partitions)
N_SU = BS // B_SUB     # 32 batch tiles per core
WINDOWS = [1, 1, 2, 4, 8, 16]  # batch windows in su units
assert sum(WINDOWS) == N_SU
N_WARM = 6
SU_W = B_SUB * IC      # elements per su per partition (1024)

F32 = mybir.dt.float32
BF16 = mybir.dt.bfloat16
FP8 = mybir.dt.float8e4

_cache = {}


def _build_program():
    nc = bacc.Bacc("TRN2", target_bir_lowering=False, debug=False,
                   num_devices=N_CORES)

    xt = nc.dram_tensor("xt", [P, N_SU * SU_W], BF16,
                        kind="ExternalInput").ap()
    wt0 = nc.dram_tensor("wt0", [P, IC * N_OC], FP8,
                         kind="ExternalInput").ap()
    wt1 = nc.dram_tensor("wt1", [P, IC * N_OC], FP8,
                         kind="ExternalInput").ap()
    bias_d = nc.dram_tensor("bias_d", [1, O_DIM], F32,
                            kind="ExternalInput").ap()
    out = nc.dram_tensor("out", [BS, O_DIM], BF16, kind="ExternalOutput").ap()

    with tile.TileContext(nc) as tc:
        with (
            tc.tile_pool(name="consts", bufs=1) as consts,
            tc.tile_pool(name="xb", bufs=1) as xb_pool,
            tc.tile_pool(name="ot", bufs=8) as ot_pool,
            tc.tile_pool(name="ps", bufs=6, space="PSUM") as ps_pool,
        ):
            # PE warmup: data-independent matmuls on scratch SBUF keep the
            # PE busy through the first window's DMA fill so HAM
            # un-throttles to 2.4 GHz before the first real matmul.
            warm_sc = consts.tile([P, N_OC], BF16)
            nc.vector.memset(warm_sc[:], 0.0)
            ps_w = ps_pool.tile([P, N_OC], F32, tag="warm", bufs=1)
            for _ in range(N_WARM):
                nc.tensor.matmul(ps_w[:], warm_sc[:, :B_SUB], warm_sc[:],
                                 start=True, stop=True, skip_group_check=True)

            # First-half weights lead the sync queue (gate of group 0);
            # second half + bias lead the scalar queue ahead of stores.
            wt_sb = []
            w_t = consts.tile([P, IC * N_OC], FP8, tag="wt0")
            nc.sync.dma_start(w_t[:], wt0[:, :])
            wt_sb.append(w_t)
            w_t = consts.tile([P, IC * N_OC], FP8, tag="wt1")
            nc.scalar.dma_start(w_t[:], wt1[:, :])
            wt_sb.append(w_t)
            bias_sb = consts.tile([P, O_DIM], F32)
            nc.scalar.dma_start(bias_sb[:],
                                bias_d[0, :].partition_broadcast(P))

            # x windows: one DMA each, enqueued up front on the sync queue.
            off = [0]
            for w in WINDOWS:
                off.append(off[-1] + w)
            xw = []
            for wi, w in enumerate(WINDOWS):
                s0 = off[wi]
                xs = xb_pool.tile([P, w * SU_W], BF16, tag=f"xs{wi}", bufs=1)
                nc.sync.dma_start(xs[:], xt[:, s0 * SU_W:(s0 + w) * SU_W])
                xw.append(xs)

            for wi, w in enumerate(WINDOWS):
                s0 = off[wi]
                for lsu in range(w):
                    su = s0 + lsu
                    r0 = su * B_SUB
                    last = su == N_SU - 1
                    ot = ot_pool.tile([P, O_DIM], BF16, tag="ot")
                    for oc in range(OC):
                        ps = ps_pool.tile([P, N_OC], F32, tag="ps")
                        for k in range(IC):
                            nc.tensor.matmul(
                                ps[:],
                                xw[wi][:, lsu * SU_W + k * B_SUB:
                                       lsu * SU_W + k * B_SUB + B_SUB],
                                wt_sb[oc][:, k * N_OC:(k + 1) * N_OC],
                                start=(k == 0),
                                stop=(k == IC - 1),
                            )
                        nc.vector.tensor_add(
                            ot[:, oc * N_OC:(oc + 1) * N_OC], ps[:],
                            bias_sb[:, oc * N_OC:(oc + 1) * N_OC])
                        if last:
                            # tail: ship each half as soon as it's ready
                            nc.scalar.dma_start(
                                out[r0:r0 + B_SUB,
                                    oc * N_OC:(oc + 1) * N_OC],
                                ot[:, oc * N_OC:(oc + 1) * N_OC])
                    if not last:
                        # 256KB fully-contiguous bf16 store of 128 rows.
                        nc.scalar.dma_start(out[r0:r0 + B_SUB, :], ot[:])

    nc.compile()
    return nc


def _get_program():
    if "prog" not in _cache:
        _cache["prog"] = _build_program()
    return _cache["prog"]


def _marshal_w(weight: np.ndarray):
    s = np.sign(weight)
    s[s == 0] = 1.0
    w3 = s.T.reshape(IC, P, O_DIM)  # [ic, p, o]
    halves = []
    for oc in range(OC):
        h = np.ascontiguousarray(
            w3[:, :, oc * N_OC:(oc + 1) * N_OC].transpose(1, 0, 2))
        halves.append(h.reshape(P, IC * N_OC).astype(ml_dtypes.float8_e4m3))
    return halves


def _marshal_x(x_shard: np.ndarray) -> np.ndarray:
    # [B, F] -> [su, b, ic, p] -> [p, su, ic, b], bf16
    x4 = x_shard.reshape(N_SU, B_SUB, IC, P).transpose(3, 0, 2, 1)
    return np.ascontiguousarray(x4).astype(ml_dtypes.bfloat16).reshape(
        P, N_SU * SU_W)


def kernel_impl(x, weight, bias, mode=None, trace=False, tmpdir=None):
    wt0, wt1 = _marshal_w(np.asarray(weight))
    bias_d = np.ascontiguousarray(np.asarray(bias, np.float32)[None, :])
    x = np.asarray(x, np.float32)

    in_maps = []
    for c in range(N_CORES):
        in_maps.append({"xt": _marshal_x(x[c * BS:(c + 1) * BS]),
                        "wt0": wt0, "wt1": wt1, "bias_d": bias_d})

    nc = _get_program()
    try:
        res = run_bass_kernel_spmd(nc, in_maps, list(range(N_CORES)),
                                   trace=trace, tmpdir=tmpdir)
    except Exception:
        # transient runtime hiccups (e.g. first dispatch after long idle)
        res = run_bass_kernel_spmd(nc, in_maps, list(range(N_CORES)),
                                   trace=trace, tmpdir=tmpdir)
    out = np.concatenate(
        [np.asarray(res.results[c]["out"]).astype(np.float32)
         for c in range(N_CORES)], axis=0)
    return out, res


def kernel(x, weight, bias):
    out, _ = kernel_impl(x, weight, bias)
    return out


# revision 9
# speedup vs baseline: 1.1116x; 1.0498x over previous
"""BinarizedLinear on 8 Trainium2 NeuronCores.

out = x @ sign(weight).T + bias
  x: (32768, 1024) f32, weight: (1024, 1024) f32, bias: (1024,) f32

Strategy (data-parallel over batch, weight/bias replicated):
  - each core handles a 4096-row shard of x
  - host marshals the shard to bf16 in a [p, su, ic, b] tiled layout
    (p = feature % 128 -> SBUF partition, su = batch/128 tile, ic =
    feature/128 contraction chunk, b = batch % 128) so that each batch
    window is ONE DMA with multi-KB contiguous per-partition segments
    (large descriptors -> near-peak HBM bandwidth) while every matmul
    stationary tile xs[.., ic*128:+128] stays a contiguous
    256B-per-partition slice (fast weight load stays enabled)
  - the binarized +-1 weight is exact in fp8, host-packed [p, ic, o']
    per output half and shipped as one 512KB DMA per half; both halves plus the
    host-replicated bias lead the scalar queue ahead of the output
    stores, so the sync queue stays pure-x and window 0 lands at full
    rate
  - device: PE matmul (x tile stationary, K=1024 accumulated in PSUM
    over 8 chunks, N=512 free) -> DVE bias-add writing bf16 ->
    contiguous 256KB store (scalar queue)
  - output returned as bf16 [4096, 1024]; host upcasts to f32
  - x windows ramp 128..2048 batch rows, all enqueued up front on the
    sync queue so DMA runs far ahead of the PE; a warmup burst
    un-throttles the PE clock (HAM) during the first window's fill
"""

import os
import sys

import numpy as np

sys.path.insert(0, "/opt/trn_rl_repo")

import ml_dtypes

import concourse.tile as tile
from concourse import bacc, mybir
from concourse.bass_utils import run_bass_kernel_spmd

N_CORES = 8
B_FULL = 32768
I_DIM = 1024
O_DIM = 1024
BS = B_FULL // N_CORES  # 4096 batch rows per core

P = 128                # partitions / contraction tile
IC = I_DIM // P        # 8 contraction chunks
N_OC = 512             # psum free width (one PSUM bank of f32)
OC = O_DIM // N_OC     # 2 output chunks
B_SUB = 128            # stationary-operand free width (psum partitions)
N_SU = BS // B_SUB     # 32 batch tiles per core
WINDOWS = [1, 1, 2, 4, 8, 16]  # batch windows in su units
assert sum(WINDOWS) == N_SU
N_WARM = 6
SU_W = B_SUB * IC      # elements per su per partition (1024)

F32 = mybir.dt.float32
BF16 = mybir.dt.bfloat16
FP8 = mybir.dt.float8e4

_cache = {}


def _build_program():
    nc = bacc.Bacc("TRN2", target_bir_lowering=False, debug=False,
                   num_devices=N_CORES)

    xt = nc.dram_tensor("xt", [P, N_SU * SU_W], BF16,
                        kind="ExternalInput").ap()
    wt0 = nc.dram_tensor("wt0", [P, IC * N_OC], FP8,
                         kind="ExternalInput").ap()
    wt1 = nc.dram_tensor("wt1", [P, IC * N_OC], FP8,
                         kind="ExternalInput").ap()
    # bias pre-replicated across partitions on the host: a plain 512KB
    # line-rate DMA instead of a 128x4KB-packet broadcast (whose per-
    # packet overhead starves the concurrent x stream)
    bias_d = nc.dram_tensor("bias_d", [P, O_DIM], F32,
                            kind="ExternalInput").ap()
    out = nc.dram_tensor("out", [BS, O_DIM], BF16, kind="ExternalOutput").ap()

    with tile.TileContext(nc) as tc:
        with (
            tc.tile_pool(name="consts", bufs=1) as consts,
            tc.tile_pool(name="xb", bufs=1) as xb_pool,
            tc.tile_pool(name="ot", bufs=8) as ot_pool,
            tc.tile_pool(name="ps", bufs=6, space="PSUM") as ps_pool,
        ):
            # PE warmup: data-independent matmuls on scratch SBUF keep the
            # PE busy through the first window's DMA fill so HAM
            # un-throttles to 2.4 GHz before the first real matmul.
            warm_sc = consts.tile([P, N_OC], BF16)
            nc.vector.memset(warm_sc[:], 0.0)
            ps_w = ps_pool.tile([P, N_OC], F32, tag="warm", bufs=1)
            for _ in range(N_WARM):
                nc.tensor.matmul(ps_w[:], warm_sc[:, :B_SUB], warm_sc[:],
                                 start=True, stop=True, skip_group_check=True)

            # Weights + bias lead the scalar queue ahead of the stores; the
            # sync queue stays pure-x so window 0 lands at full rate.
            wt_sb = []
            for oc, wsrc in enumerate((wt0, wt1)):
                w_t = consts.tile([P, IC * N_OC], FP8, tag=f"wt{oc}")
                nc.scalar.dma_start(w_t[:], wsrc[:, :])
                wt_sb.append(w_t)
            bias_sb = consts.tile([P, O_DIM], F32)
            nc.scalar.dma_start(bias_sb[:], bias_d[:, :])

            # x windows: one DMA each, enqueued up front on the sync queue.
            off = [0]
            for w in WINDOWS:
                off.append(off[-1] + w)
            xw = []
            for wi, w in enumerate(WINDOWS):
                s0 = off[wi]
                xs = xb_pool.tile([P, w * SU_W], BF16, tag=f"xs{wi}", bufs=1)
                nc.sync.dma_start(xs[:], xt[:, s0 * SU_W:(s0 + w) * SU_W])
                xw.append(xs)

            for wi, w in enumerate(WINDOWS):
                s0 = off[wi]
                for lsu in range(w):
                    su = s0 + lsu
                    r0 = su * B_SUB
                    last = su == N_SU - 1
                    ot = ot_pool.tile([P, O_DIM], BF16, tag="ot")
                    for oc in range(OC):
                        ps = ps_pool.tile([P, N_OC], F32, tag="ps")
                        for k in range(IC):
                            nc.tensor.matmul(
                                ps[:],
                                xw[wi][:, lsu * SU_W + k * B_SUB:
                                       lsu * SU_W + k * B_SUB + B_SUB],
                                wt_sb[oc][:, k * N_OC:(k + 1) * N_OC],
                                start=(k == 0),
                                stop=(k == IC - 1),
                            )
                        nc.vector.tensor_add(
                            ot[:, oc * N_OC:(oc + 1) * N_OC], ps[:],
                            bias_sb[:, oc * N_OC:(oc + 1) * N_OC])
                        if last:
                            # tail: ship each half as soon as it's ready
                            nc.scalar.dma_start(
                                out[r0:r0 + B_SUB,
                                    oc * N_OC:(oc + 1) * N_OC],
                                ot[:, oc * N_OC:(oc + 1) * N_OC])
                    if not last:
                        # 256KB fully-contiguous bf16 store of 128 rows.
                        nc.scalar.dma_start(out[r0:r0 + B_SUB, :], ot[:])

    nc.compile()
    return nc


def _get_program():
    if "prog" not in _cache:
        _cache["prog"] = _build_program()
    return _cache["prog"]


def _marshal_w(weight: np.ndarray):
    s = np.sign(weight)
    s[s == 0] = 1.0
    w3 = s.T.reshape(IC, P, O_DIM)  # [ic, p, o]
    halves = []
    for oc in range(OC):
        h = np.ascontiguousarray(
            w3[:, :, oc * N_OC:(oc + 1) * N_OC].transpose(1, 0, 2))
        halves.append(h.reshape(P, IC * N_OC).astype(ml_dtypes.float8_e4m3))
    return halves


def _marshal_x(x_shard: np.ndarray) -> np.ndarray:
    # [B, F] -> [su, b, ic, p] -> [p, su, ic, b], bf16
    x4 = x_shard.reshape(N_SU, B_SUB, IC, P).transpose(3, 0, 2, 1)
    return np.ascontiguousarray(x4).astype(ml_dtypes.bfloat16).reshape(
        P, N_SU * SU_W)


def kernel_impl(x, weight, bias, mode=None, trace=False, tmpdir=None):
    wt0, wt1 = _marshal_w(np.asarray(weight))
    bias_d = np.ascontiguousarray(
        np.broadcast_to(np.asarray(bias, np.float32)[None, :], (P, O_DIM)))
    x = np.asarray(x, np.float32)

    in_maps = []
    for c in range(N_CORES):
        in_maps.append({"xt": _marshal_x(x[c * BS:(c + 1) * BS]),
                        "wt0": wt0, "wt1": wt1, "bias_d": bias_d})

    nc = _get_program()
    try:
        res = run_bass_kernel_spmd(nc, in_maps, list(range(N_CORES)),
                                   trace=trace, tmpdir=tmpdir)
    except Exception:
        # transient runtime hiccups (e.g. first dispatch after long idle)
        res = run_bass_kernel_spmd(nc, in_maps, list(range(N_CORES)),
                                   trace=trace, tmpdir=tmpdir)
    out = np.concatenate(
        [np.asarray(res.results[c]["out"]).astype(np.float32)
         for c in range(N_CORES)], axis=0)
    return out, res


def kernel(x, weight, bias):
    out, _ = kernel_impl(x, weight, bias)
    return out
